# revision 1
# baseline (speedup 1.0000x reference)
"""Trainium2 Bass kernel for nn_AttentionLayer (attention pooling).

Reference math (per batch row b):
    u   = tanh(x[b] @ W + b_vec)        # [T, M]
    s   = u @ us                        # [T]
    a   = softmax(s) * mask / sum       # [T]  (mask is all ones per spec)
    out = a @ x[b]                      # [D]

Strategy: data-parallel over batch, B=32 rows -> 4 rows per NeuronCore on
8 cores.  Per core, per row, in bf16 on the TensorEngine:
  - x tiles [128t, 1024d] are DMA'd in fp32, cast to bf16 (DVE), and
    PE-transposed (128x128 blocks via identity matmuls) into x^T layout;
  - u^T = tanh(W^T x^T + bias) accumulates in PSUM, tanh fused on ScalarE;
  - scores come out pre-transposed ([t,1] per 128-chunk) via
    matmul(lhsT=u^T chunk, rhs=us);
  - exp on ScalarE (no max subtraction needed: |s| <= ||us||_1 ~ 5, exact
    softmax up to fp32 rounding), per-partition sums via ACT accum_out;
  - out = (e^T x) / sum(e): two PE column-groups compute the two d-halves
    concurrently into separate PSUM banks.
The emission is software-pipelined so each quarter-row's matmuls drain
interleaved with the next quarter's transposes.
"""
import numpy as np

import concourse.bacc as bacc
import concourse.mybir as mybir
from concourse.tile import TileContext
from concourse.masks import make_identity
from concourse.bass_utils import run_bass_kernel_spmd

F32 = mybir.dt.float32
BF16 = mybir.dt.bfloat16

B, T, D, M = 32, 2048, 1024, 128
NCORES = 8
B_SH = B // NCORES   # 4 batch rows per core
P = 128
NT = T // P          # 16 t-tiles per row
NCD = D // P         # 8 d-chunks
QT = 4               # t-tiles per quarter-row
NQ = NT // QT        # 4 quarters per row


def _build_nc():
    nc = bacc.Bacc("TRN2", target_bir_lowering=False, debug=False,
                   num_devices=NCORES)
    x = nc.declare_dram_parameter("x", [B_SH, T, D], F32, isOutput=False)
    W = nc.declare_dram_parameter("W", [D, M], F32, isOutput=False)
    b = nc.declare_dram_parameter("b", [M], F32, isOutput=False)
    us = nc.declare_dram_parameter("us", [M, 1], F32, isOutput=False)
    y = nc.declare_dram_parameter("y", [B_SH, D], F32, isOutput=True)

    with TileContext(nc) as tc:
        with (
            tc.tile_pool(name="singles", bufs=1) as singles,
            tc.tile_pool(name="stage", bufs=4) as stage,
            tc.tile_pool(name="xb", bufs=2) as xb_pool,
            tc.tile_pool(name="xt", bufs=3) as xt_pool,
            tc.tile_pool(name="u", bufs=2) as u_pool,
            tc.tile_pool(name="et", bufs=4) as et_pool,
            tc.tile_pool(name="outs", bufs=2) as out_pool,
            tc.tile_pool(name="tp_ps", bufs=3, space="PSUM") as tp_psum,
            tc.tile_pool(name="u_ps", bufs=2, space="PSUM") as u_psum,
            tc.tile_pool(name="s_ps", bufs=1, space="PSUM") as s_psum,
            tc.tile_pool(name="o_ps", bufs=2, space="PSUM") as o_psum,
        ):
            # warm-up constant on DVE only (no gpsimd library wait)
            wones = singles.tile([P, P], BF16)
            nc.vector.memset(wones, 1.0)
            ones_f32 = singles.tile([P, 1], F32)
            nc.vector.memset(ones_f32, 1.0)

            # kick off the first row's DMAs before anything else
            stage_tiles = {}
            for q in range(NQ):
                st = stage.tile([P, QT, D], F32, tag="stage")
                nc.sync.dma_start(
                    out=st,
                    in_=x[0].rearrange("(n p) d -> p n d", p=P)[:, q * QT:(q + 1) * QT, :],
                )
                stage_tiles[(0, q)] = st

            # PE warm-up: get HAM to K=8/8 while the first DMAs stream
            warm = u_psum.tile([P, QT * P], F32, tag="up")
            for i in range(56):
                nc.tensor.matmul(warm[:, :P], wones, wones, start=True, stop=True)

            # constants
            w_f32 = singles.tile([P, NCD, M], F32)
            nc.sync.dma_start(out=w_f32, in_=W.rearrange("(c p) m -> p c m", p=P))
            w_bf = singles.tile([P, NCD, M], BF16)
            nc.vector.tensor_copy(out=w_bf, in_=w_f32)

            b_sb = singles.tile([P, 1], F32)
            nc.sync.dma_start(out=b_sb, in_=b.rearrange("(p o) -> p o", o=1))

            us_f32 = singles.tile([P, 1], F32)
            nc.sync.dma_start(out=us_f32, in_=us[:, :])
            us_bf = singles.tile([P, 1], BF16)
            nc.vector.tensor_copy(out=us_bf, in_=us_f32)

            ident = singles.tile([P, P], BF16)
            make_identity(nc, ident)

            # software pipeline over all quarters
            rowstate = {}
            compute1 = {}
            compute2 = {}

            def drain(ops, k):
                for _ in range(k):
                    if ops:
                        ops.pop(0)()

            for idx in range(B_SH * NQ):
                r, q = divmod(idx, NQ)
                if q == 0:
                    rowstate[r] = dict(
                        x_bf=xb_pool.tile([P, NT, D], BF16, tag="xb", name=f"x_bf_{r}"),
                        u_sb=u_pool.tile([P, T], BF16, tag="u", name=f"u_sb_{r}"),
                        rs=out_pool.tile([P, NQ], F32, tag="rs", name=f"rs_{r}"),
                        opa=o_psum.tile([P, 512], F32, tag="o", name=f"opa_{r}"),
                        opb=o_psum.tile([P, 512], F32, tag="o", name=f"opb_{r}"),
                        o_sb=out_pool.tile([1, D], F32, tag="o_sb", name=f"o_sb_{r}"),
                    )
                rs_ = rowstate[r]
                x_bf, u_sb = rs_["x_bf"], rs_["u_sb"]

                if (r, q) in stage_tiles:
                    st = stage_tiles.pop((r, q))
                else:
                    st = stage.tile([P, QT, D], F32, tag="stage")
                    nc.sync.dma_start(
                        out=st,
                        in_=x[r].rearrange("(n p) d -> p n d", p=P)[:, q * QT:(q + 1) * QT, :],
                    )
                nc.vector.tensor_copy(out=x_bf[:, q * QT:(q + 1) * QT, :], in_=st)

                # transposes for this quarter, interleaved with draining the
                # previous quarters' matmul work
                xt = xt_pool.tile([P, QT, NCD, P], BF16, tag="xt")
                c1 = compute1.pop(idx - 1, [])
                c2 = compute2.pop(idx - 2, [])
                for j in range(QT):
                    t_idx = q * QT + j
                    for g in range(2):
                        tp = tp_psum.tile([P, 4 * P], BF16, tag="tp")
                        for cc in range(4):
                            c = g * 4 + cc
                            nc.tensor.transpose(
                                tp[:, cc * P:(cc + 1) * P],
                                x_bf[:, t_idx, c * P:(c + 1) * P],
                                ident,
                            )
                        dst = xt[:, j, g * 4:(g + 1) * 4, :]
                        src = tp.rearrange("p (c t) -> p c t", c=4)
                        if (2 * j + g) % 2 == 0:
                            nc.scalar.copy(out=dst, in_=src)
                        else:
                            nc.vector.tensor_copy(out=dst, in_=src)
                        drain(c1, 2)
                        drain(c2, 1)
                drain(c1, len(c1))
                drain(c2, len(c2))

                def make_c1(r=r, q=q, xt=xt, u_sb=u_sb, rs_=rs_):
                    ops = []
                    up = u_psum.tile([P, QT * P], F32, tag="up")

                    def mk_p1(c):
                        def f():
                            nc.tensor.matmul(
                                up, w_bf[:, c, :], xt[:, :, c, :],
                                start=(c == 0), stop=(c == NCD - 1),
                            )
                        return f
                    for c in range(NCD):
                        ops.append(mk_p1(c))

                    def tanh_op():
                        nc.scalar.activation(
                            out=u_sb[:, q * QT * P:(q + 1) * QT * P], in_=up,
                            func=mybir.ActivationFunctionType.Tanh,
                            bias=b_sb, scale=1.0,
                        )
                    ops.append(tanh_op)

                    sp = s_psum.tile([P, QT], F32, tag="s")

                    def mk_st(j):
                        def f():
                            t_idx = q * QT + j
                            nc.tensor.matmul(
                                sp[:, j:j + 1],
                                u_sb[:, t_idx * P:(t_idx + 1) * P],
                                us_bf, start=True, stop=True,
                            )
                        return f
                    for j in range(QT):
                        ops.append(mk_st(j))

                    etq = et_pool.tile([P, QT], BF16, tag="et")
                    rs_[f"et{q}"] = etq

                    def exp_op():
                        nc.scalar.activation(
                            out=etq, in_=sp,
                            func=mybir.ActivationFunctionType.Exp,
                            accum_out=rs_["rs"][:, q:q + 1],
                        )
                    ops.append(exp_op)
                    return ops

                compute1[idx] = make_c1()

                def make_c2(r=r, q=q, x_bf=x_bf, rs_=rs_):
                    ops = []

                    def mk_p2(j, g):
                        def f():
                            t_idx = q * QT + j
                            op_t = rs_["opa"] if g == 0 else rs_["opb"]
                            kwargs = {}
                            if g == 1:
                                kwargs["tile_position"] = (0, 64)
                            nc.tensor.matmul(
                                op_t[64 * g:64 * g + 1, :],
                                rs_[f"et{q}"][:, j:j + 1],
                                x_bf[:, t_idx, g * 512:(g + 1) * 512],
                                start=(q == 0 and j == 0),
                                stop=(q == NQ - 1 and j == QT - 1),
                                **kwargs,
                            )
                        return f
                    for j in range(QT):
                        for g in range(2):
                            ops.append(mk_p2(j, g))

                    if q == NQ - 1:
                        def finish():
                            dnp = s_psum.tile([1, NQ], F32, tag="s")
                            nc.tensor.matmul(dnp, ones_f32, rs_["rs"],
                                             start=True, stop=True)
                            dsum = out_pool.tile([1, 1], F32, tag="dsum")
                            nc.vector.reduce_sum(out=dsum, in_=dnp,
                                                 axis=mybir.AxisListType.X)
                            inv = out_pool.tile([1, 1], F32, tag="inv")
                            nc.vector.reciprocal(out=inv, in_=dsum)
                            o_sb = rs_["o_sb"]
                            nc.vector.tensor_scalar_mul(
                                o_sb[:, 0:512], rs_["opa"][0:1, :], inv)
                            nc.vector.tensor_scalar_mul(
                                o_sb[:, 512:1024], rs_["opb"][64:65, :], inv)
                            nc.sync.dma_start(out=y[r:r + 1, :], in_=o_sb)
                        ops.append(finish)
                    return ops

                compute2[idx] = make_c2()

            for idx in sorted(set(compute1) | set(compute2)):
                for f in compute1.pop(idx, []):
                    f()
                for f in compute2.pop(idx, []):
                    f()

    nc.compile()
    return nc


_NC_CACHE = []


def _numpy_reference(x, W, b, us, mask):
    m = mask.astype(x.dtype)
    u = np.tanh(np.einsum('btd,dm->btm', x, W) + b)
    utu = np.einsum('btm,mo->bto', u, us)[..., 0]
    e = np.exp(utu - utu.max(axis=-1, keepdims=True))
    e = m * e
    a = e / e.sum(axis=-1, keepdims=True)
    return np.einsum('bt,btd->bd', a, x).astype(np.float32)


def kernel(x, W, b, us, mask):
    x = np.ascontiguousarray(np.asarray(x, dtype=np.float32))
    W = np.ascontiguousarray(np.asarray(W, dtype=np.float32))
    b = np.ascontiguousarray(np.asarray(b, dtype=np.float32))
    us = np.ascontiguousarray(np.asarray(us, dtype=np.float32))
    mask = np.asarray(mask)

    if not bool(mask.all()):
        # spec guarantees an all-ones mask; fall back to exact numpy
        # reference if that ever changes
        return _numpy_reference(x, W, b, us, mask)

    if not _NC_CACHE:
        _NC_CACHE.append(_build_nc())
    nc = _NC_CACHE[0]

    in_maps = []
    for i in range(NCORES):
        in_maps.append({
            "x": np.ascontiguousarray(x[i * B_SH:(i + 1) * B_SH]),
            "W": W, "b": b, "us": us,
        })
    res = run_bass_kernel_spmd(nc, in_maps, core_ids=list(range(NCORES)),
                               trace=False)
    return np.concatenate([res.results[i]["y"] for i in range(NCORES)], axis=0)



# revision 4
# speedup vs baseline: 1.4674x; 1.4674x over previous
"""Trainium2 Bass kernel for nn_AttentionLayer (attention pooling).

Reference math (per batch row b):
    u   = tanh(x[b] @ W + b_vec)        # [T, M]
    s   = u @ us                        # [T]
    a   = softmax(s) * mask / sum       # [T]  (mask is all ones per spec)
    out = a @ x[b]                      # [D]

Strategy: data-parallel over batch, B=32 rows -> 4 rows per NeuronCore on
8 cores.  x is converted to bf16 on the HOST (the device math is bf16
anyway), which halves HBM traffic and removes the on-device cast.
Per core, per row:
  - x tiles [128t, 1024d] are DMA'd bf16 and PE-transposed (LDW+MM via
    identity, back-to-back pairs) into x^T layout; DVE copies PSUM->SBUF;
  - u^T = tanh(W^T x^T + bias) accumulates in PSUM with the W-chunk
    stationary reused across a half-row (c-outer loop), tanh on ScalarE;
  - scores per t-chunk via matmul(lhsT=u chunk, rhs=us) -> [128t, 1];
  - exp on ScalarE with accum_out row sums; Sum(e) is reduced by a
    ones-matmul, inverted on DVE, broadcast back through a 1xK matmul,
    and e is pre-scaled by 1/Sum(e) so pooling emits normalized output;
  - pooling = four COLUMN-TILED matmuls per t-chunk (tile_position
    (0,32g), N=256 each) that stream concurrently on separate XBUSes;
    output goes straight from PSUM to DRAM.
Pooling of row r is emitted during row r+1 so the PE queue never blocks
on the exp/normalize chain.
"""
import numpy as np
import ml_dtypes

import concourse.bacc as bacc
import concourse.mybir as mybir
from concourse.tile import TileContext
from concourse.masks import make_identity
from concourse.bass_utils import run_bass_kernel_spmd

F32 = mybir.dt.float32
BF16 = mybir.dt.bfloat16

B, T, D, M = 32, 2048, 1024, 128
NCORES = 8
B_SH = B // NCORES   # 4 batch rows per core
P = 128
NT = T // P          # 16 t-tiles per row
NCD = D // P         # 8 d-chunks of 128
NH = 2               # half-rows
TPH = NT // NH       # 8 t-tiles per half-row
DG = D // 4          # 256 columns per pooling col-group


def _build_nc():
    nc = bacc.Bacc("TRN2", target_bir_lowering=False, debug=False,
                   num_devices=NCORES)
    x = nc.declare_dram_parameter("x", [B_SH, T, D], BF16, isOutput=False)
    W = nc.declare_dram_parameter("W", [D, M], BF16, isOutput=False)
    b = nc.declare_dram_parameter("b", [M], F32, isOutput=False)
    us = nc.declare_dram_parameter("us", [M, 1], BF16, isOutput=False)
    y = nc.declare_dram_parameter("y", [B_SH, D], F32, isOutput=True)

    with TileContext(nc) as tc:
        with (
            tc.tile_pool(name="singles", bufs=1) as singles,
            tc.tile_pool(name="xb", bufs=3) as xb_pool,
            tc.tile_pool(name="xt", bufs=2) as xt_pool,
            tc.tile_pool(name="u", bufs=2) as u_pool,
            tc.tile_pool(name="e", bufs=2) as e_pool,
            tc.tile_pool(name="tp_ps", bufs=3, space="PSUM") as tp_psum,
            tc.tile_pool(name="u_ps", bufs=2, space="PSUM") as u_psum,
            tc.tile_pool(name="s_ps", bufs=1, space="PSUM") as s_psum,
            tc.tile_pool(name="o_ps", bufs=2, space="PSUM") as o_psum,
        ):
            # constants (DVE memsets only; no gpsimd library wait)
            wones = singles.tile([P, P], BF16)
            nc.vector.memset(wones, 1.0)
            ones_col = singles.tile([P, 1], F32)
            nc.vector.memset(ones_col, 1.0)
            ones_row = singles.tile([1, P], F32)
            nc.vector.memset(ones_row, 1.0)

            # first row's x DMAs before anything else
            x_tiles = {}

            def start_row_dmas(r):
                xb = xb_pool.tile([P, NT, D], BF16, tag="xb", name=f"xb_{r}")
                src = x[r].rearrange("(n p) d -> p n d", p=P)
                for t in range(NT):
                    nc.sync.dma_start(out=xb[:, t, :], in_=src[:, t, :])
                x_tiles[r] = xb

            start_row_dmas(0)

            # small constants
            w_bf = singles.tile([P, NCD, M], BF16)
            nc.sync.dma_start(out=w_bf, in_=W.rearrange("(c p) m -> p c m", p=P))
            b_sb = singles.tile([P, 1], F32)
            nc.sync.dma_start(out=b_sb, in_=b.rearrange("(p o) -> p o", o=1))
            us_bf = singles.tile([P, 1], BF16)
            nc.sync.dma_start(out=us_bf, in_=us[:, :])
            ident = singles.tile([P, P], BF16)
            make_identity(nc, ident)

            # PE warm-up while the first DMAs stream (HAM un-throttle)
            warm = u_psum.tile([P, 4, P], F32, tag="up")
            for _ in range(24):
                nc.tensor.matmul(warm[:, 0, :], wones, wones,
                                 start=True, stop=True)

            pending_pool = []   # emits pooling of row r during row r+1

            def emit_pooling(r, e_scaled, o_tile):
                for j in range(NT):
                    for g in range(4):
                        nc.tensor.matmul(
                            o_tile[32 * g:32 * g + 1, :DG],
                            e_scaled[:, j:j + 1],
                            x_tiles[r][:, j, g * DG:(g + 1) * DG],
                            start=(j == 0), stop=(j == NT - 1),
                            tile_position=(0, 32 * g),
                        )
                o_sb = e_pool.tile([P, DG], F32, tag="osb", name=f"osb_{r}")
                for g in range(4):
                    nc.vector.tensor_copy(
                        out=o_sb[32 * g:32 * g + 1, :],
                        in_=o_tile[32 * g:32 * g + 1, :DG],
                    )
                    nc.sync.dma_start(
                        out=y[r:r + 1, g * DG:(g + 1) * DG],
                        in_=o_sb[32 * g:32 * g + 1, :],
                    )

            for r in range(B_SH):
                xb = x_tiles[r]
                if r + 1 < B_SH:
                    start_row_dmas(r + 1)

                u_sb = u_pool.tile([P, T], BF16, tag="u", name=f"u_{r}")
                sp = s_psum.tile([P, NT + 2], F32, tag="s")
                rs = e_pool.tile([P, 1], F32, tag="rs", name=f"rs_{r}")

                for h in range(NH):
                    # transposes for this half-row
                    xt = xt_pool.tile([P, TPH, NCD, P], BF16, tag="xt")
                    for tt in range(TPH):
                        t = h * TPH + tt
                        tp = tp_psum.tile([P, NCD, P], BF16, tag="tp")
                        for c in range(NCD):
                            nc.tensor.transpose(
                                tp[:, c, :],
                                xb[:, t, c * P:(c + 1) * P],
                                ident,
                            )
                        nc.vector.tensor_copy(out=xt[:, tt, :, :], in_=tp)

                    # row r-1 pooling slots in here (after first half's
                    # transposes) so PE stays busy while it has slack
                    if h == 1 and pending_pool:
                        emit_pooling(*pending_pool.pop(0))

                    # GEMM: c-outer, W-chunk stationary reused for both
                    # quarter streams of this half-row
                    ups = [u_psum.tile([P, 4, P], F32, tag="up", name=f"up{q}")
                           for q in range(2)]
                    for c in range(NCD):
                        for q in range(2):
                            nc.tensor.matmul(
                                ups[q], w_bf[:, c, :],
                                xt[:, 4 * q:4 * q + 4, c, :],
                                start=(c == 0), stop=(c == NCD - 1),
                            )
                    for q in range(2):
                        qg = h * 2 + q
                        nc.scalar.activation(
                            out=u_sb[:, qg * 512:(qg + 1) * 512],
                            in_=ups[q],
                            func=mybir.ActivationFunctionType.Tanh,
                            bias=b_sb, scale=1.0,
                        )
                        for jj in range(4):
                            t = qg * 4 + jj
                            nc.tensor.matmul(
                                sp[:, t:t + 1],
                                u_sb[:, t * P:(t + 1) * P],
                                us_bf, start=True, stop=True,
                            )

                # softmax weights, pre-scaled by 1/sum
                e_pack = e_pool.tile([P, NT], BF16, tag="ep", name=f"ep_{r}")
                nc.scalar.activation(
                    out=e_pack, in_=sp[:, :NT],
                    func=mybir.ActivationFunctionType.Exp,
                    accum_out=rs,
                )
                nc.tensor.matmul(sp[0:1, NT:NT + 1], rs, ones_col,
                                 start=True, stop=True)
                tinv = e_pool.tile([1, 1], F32, tag="tinv", name=f"tinv_{r}")
                nc.vector.reciprocal(out=tinv, in_=sp[0:1, NT:NT + 1])
                nc.tensor.matmul(sp[:, NT + 1:NT + 2], ones_row, tinv,
                                 start=True, stop=True)
                e_scaled = e_pool.tile([P, NT], BF16, tag="es", name=f"es_{r}")
                nc.vector.tensor_scalar_mul(e_scaled, e_pack,
                                            sp[:, NT + 1:NT + 2])

                o_tile = o_psum.tile([P, DG], F32, tag="o")
                pending_pool.append((r, e_scaled, o_tile))

            while pending_pool:
                emit_pooling(*pending_pool.pop(0))

    nc.compile()
    return nc


_NC_CACHE = []


def _numpy_reference(x, W, b, us, mask):
    m = mask.astype(x.dtype)
    u = np.tanh(np.einsum('btd,dm->btm', x, W) + b)
    utu = np.einsum('btm,mo->bto', u, us)[..., 0]
    e = np.exp(utu - utu.max(axis=-1, keepdims=True))
    e = m * e
    a = e / e.sum(axis=-1, keepdims=True)
    return np.einsum('bt,btd->bd', a, x).astype(np.float32)


def _make_in_maps(x, W, b, us):
    x_bf = np.ascontiguousarray(x).astype(ml_dtypes.bfloat16)
    W_bf = np.ascontiguousarray(W).astype(ml_dtypes.bfloat16)
    us_bf = np.ascontiguousarray(us).astype(ml_dtypes.bfloat16)
    b_f = np.ascontiguousarray(b).astype(np.float32)
    in_maps = []
    for i in range(NCORES):
        in_maps.append({
            "x": np.ascontiguousarray(x_bf[i * B_SH:(i + 1) * B_SH]),
            "W": W_bf, "b": b_f, "us": us_bf,
        })
    return in_maps


def kernel(x, W, b, us, mask):
    x = np.asarray(x, dtype=np.float32)
    W = np.asarray(W, dtype=np.float32)
    b = np.asarray(b, dtype=np.float32)
    us = np.asarray(us, dtype=np.float32)
    mask = np.asarray(mask)

    if not bool(mask.all()):
        # spec guarantees an all-ones mask; fall back to exact numpy
        # reference if that ever changes
        return _numpy_reference(x, W, b, us, mask)

    if not _NC_CACHE:
        _NC_CACHE.append(_build_nc())
    nc = _NC_CACHE[0]

    res = run_bass_kernel_spmd(nc, _make_in_maps(x, W, b, us),
                               core_ids=list(range(NCORES)), trace=False)
    return np.concatenate([res.results[i]["y"] for i in range(NCORES)], axis=0)
